# revision 76
# baseline (speedup 1.0000x reference)
"""Trainium2 Bass kernel for nn_AttentionLayer_48722109006175.

Math: out[b,i,j,h] = softmax_h( leaky_relu( s[b,i,h] + d[b,j,h]
                                            + cnt[i,j]*(g[b,i,h]-g[b,j,h]), 0.2 ) )

with s = src@(W_lin@a_src), d = src@(W_lin@a_dst), g = src@(W_edge@a_edge)
and cnt[i,j] the (batch-independent) edge multiplicity matrix.

All three logit contributions accumulate on the PE into one PSUM tile
psa [128, 4*512] (head-blocks of 512 j's), per (i-tile, j-block):
  P:    psa[h] += s_h[i] + d_h[j]          rank-2 f32r matmul (free 512)
  row:  psa[h] += g_h[i]*cnt[i,j]          diag(g_h[i-tile]) @ cnt   (bf16)
  col:  psa[h] -= g_h[j]*cnt[i,j]          (-cnt^T chunk) @ diag-pair (bf16,
        one matmul per 128-j chunk covering all 4 heads via a strided
        PSUM out AP)
All are 1 cycle/row on PE, so the edge scatter costs the same as the
rank-2 part and no dense DVE cnt-multiply pass is needed.

The elementwise tail is software-pipelined across tiles (stages lag
1-4 tiles so no in-order engine queue ever head-of-line blocks on a
cross-engine dependency):
  A(t):   PE accumulate + prelu (a registered single-input custom-DVE
          op max(x, 0.2x) on ~half the tiles, ACT Prelu on the rest)
  B(t-1): ACT exp -> m bf16
  E(t-2): DVE pair-sum (bf16 2x mode)
  C(t-3): custom-DVE fused sum+reciprocal (seed + one Newton step,
          +-0.17% - below the bf16 write quantization), bf16 out
  D(t-4): DVE broadcast multiply (bf16 2x mode, half offloaded to
          Pool except for the last tiles) + store
Output is stored bf16 in head-plane layout [H,N,N]; the host
transposes to [N,N,H] f32.  cnt and -cnt^T (packed per-i-tile) ship
as one concatenated bf16 tensor, one load per i-tile (all 8 stay
resident; the j-block-major tile order reuses them).

Startup latency: the prologue runs in N/2 halves (src load ->
transpose -> sdg -> staging rows), with the src and first cnt DMAs
leading the SP queue and ones/staging broadcasts on the Pool queue,
so the first i-tile pass starts ~11us in, needing only half-A
staging.  psa is split into two 2-bank half-tiles (head pairs) so
PE's psa reuse only waits on the matching half's prelu.  One quarter
of the final multiply runs on the Pool engine (except for the last
tiles, where Pool would straggle the closing store).
Sharding: data-parallel over batch, one batch per NeuronCore.
Measured: 70592 ns (cost-model timeline), absmax rel err 9.6e-3
(vs 122886 ns baseline, 1.74x).
"""

import numpy as np

B, N, F_IN, H = 8, 1024, 128, 4
JB = 512          # j-block
NT = N // 128     # 8 i-tiles
NC = N // 128     # 8 j-chunks of 128
NEG_SLOPE = 0.2


def _leaky_relu_dve_op():
    """Register (once) a single-input custom-DVE op computing
    out = max(x, NEG_SLOPE*x).  A plain scalar_tensor_tensor(psa, c, psa)
    reads PSUM twice, which the DVE forbids; this op reads Src0 once.
    Registration follows the documented extension path in dve_ops.py
    (append to OPS + the name->row map); the per-NEFF uop table is then
    generated by the normal compile_bir_kernel flow."""
    import numpy as np
    import concourse.dve_ops as dve_ops
    from concourse.dve_spec import Spec, Src0, C2, maxx, lower, _has_src1
    from concourse.dve_uop import DveOpSpec

    NAME = "PRELU_LEAKY_ANT"
    for op in dve_ops.OPS:
        if op.name == NAME:
            return op
    spec = Spec(
        body=maxx(Src0, Src0 * C2),
        reference=lambda in0, in1, s0, s1, imm2: np.maximum(
            in0, in0 * imm2).astype(np.float32),
    )
    row = max(dve_ops._SUB_OPCODE_FOR_NAME.values()) + 1
    assert row < 0x20
    shas = {}
    for ver in ("v3", "v4"):
        compiled = DveOpSpec(name=NAME, opcode=row, uops=lower(spec, ver=ver),
                             rd1_en=_has_src1(spec))
        shas[ver] = compiled.sha(ver)
    op = dve_ops.DveOp(NAME, spec, subdim=False, uops_sha=shas)
    dve_ops.OPS.append(op)
    dve_ops._SUB_OPCODE_FOR_NAME[NAME] = row
    dve_ops.CUSTOM_DVE_SPECS[NAME] = spec
    return op


def _recip_sum_dve_op():
    """Register (once) a custom-DVE op: out = approx 1/(Src0+Src1).

    BITWISE_NOT exponent-flip seed + ONE inline Newton pass with the
    RECIP_APPROX_FAST Chebyshev pair (those constants equioscillate the
    post-NR error at ~0.17%, below the bf16 quantization of the result,
    so the second NR pass of the library op buys nothing here).  Fusing
    the final head-pair sum into the reciprocal keeps the whole softmax
    normalize on the DVE - no Pool round trip per tile."""
    import numpy as np
    import concourse.dve_ops as dve_ops
    from concourse.dve_spec import (Spec, Src0, Src1, C0, C1, AluOp, Bin,
                                    lower, _has_src1)
    from concourse.dve_uop import DveOpSpec

    NAME = "RECIP_SUM_ANT"
    for op in dve_ops.OPS:
        if op.name == NAME:
            return op

    S = Src0 + Src1
    nx = Bin(AluOp.BITWISE_NOT, S, S)
    y0 = nx * C0
    body = y0 * (C1 - S * y0)

    def ref(in0, in1, s0, s1, imm2):
        s = (np.asarray(in0, np.float32) + np.asarray(in1, np.float32))
        not_x = (~s.view(np.int32)).view(np.float32)
        y0 = not_x * s0
        return (y0 * (s1 - s * y0)).astype(np.float32)

    spec = Spec(body=body, reference=ref)
    row = max(dve_ops._SUB_OPCODE_FOR_NAME.values()) + 1
    assert row < 0x20
    shas = {}
    for ver in ("v3", "v4"):
        compiled = DveOpSpec(name=NAME, opcode=row, uops=lower(spec, ver=ver),
                             rd1_en=_has_src1(spec))
        shas[ver] = compiled.sha(ver)
    op = dve_ops.DveOp(NAME, spec, subdim=False, uops_sha=shas)
    dve_ops.OPS.append(op)
    dve_ops._SUB_OPCODE_FOR_NAME[NAME] = row
    dve_ops.CUSTOM_DVE_SPECS[NAME] = spec
    return op


CFG = {
    "dve_prelu": (1, 2, 4, 7, 8),  # t%9 residues routed to DVE prelu
    "store_per_tile": False,      # (unused in staged pipeline)
    "lp": 4, "mp": 7, "ob": 4, "mn": 6, "cnt": 8,
    "redo0": False,               # recompute i-tile 0 at the end
    "ablate": 0,                  # 0=full .. 6=PE only (debug)
    "stage_order": "abecd",
    "divide": False,
    "pool_s2": (),
    "pool_mult": 2,
    "pm_cut": 13,
    "s_dve_from": 99,
    "fuse_rsum": True,
    "dve_copies": True,
    "ps2_from": 99,
    "dve_prelu_until": 99,
    "dve_srct": False,
    "split_last": 2,
    "ppt": 2,
    "inplace_prelu": False,
}


def _build_nc():
    import concourse.bass as bass
    import concourse.bacc as bacc
    import concourse.mybir as mybir
    import concourse.tile as tile
    from concourse.masks import make_identity

    prelu_op = _leaky_relu_dve_op()
    rsum_op = _recip_sum_dve_op()

    f32 = mybir.dt.float32
    f32r = mybir.dt.float32r
    bf16 = mybir.dt.bfloat16
    AF = mybir.ActivationFunctionType
    OP = mybir.AluOpType

    nc = bacc.Bacc()
    # Reset DMA queues + clear bass-managed semaphores at kernel entry.
    # (Bass only emits this when target_bir_lowering=True; without it, stale
    # semaphore/DMA state from previously-executed NEFFs on the same core
    # races the first tile loads.)
    from concourse.bass import compact_to_ranges
    for sem_range in compact_to_ranges(
        [s for s in nc._kernel_sem_range if s not in nc.barrier_sems]
    ):
        nc.gpsimd.dma_reset(sem_range)
        nc.gpsimd.sem_clear(sem_range)
    nc._nrt_pseudo_barrier()

    src_d = nc.dram_tensor("src", [N, F_IN], f32, kind="ExternalInput")
    # rows 0..N: cnt (bf16); rows N..2N: -cnt^T packed per-i-tile slab
    cn_d = nc.dram_tensor("cn", [2 * N, N], bf16, kind="ExternalInput")
    a_d = nc.dram_tensor("A", [F_IN, 12], f32, kind="ExternalInput")
    out_d = nc.dram_tensor("out", [H, N, N], bf16, kind="ExternalOutput")

    with tile.TileContext(nc) as tc:
        with tc.tile_pool(name="stage", bufs=1) as stage:
            # P-matmul staging, split by N/2 halves so the first j-block
            # pass only waits on the half-A prologue chain:
            # lhs row0 = [s_0..s_3] 512-chunks, row1 = ones;
            # rhs row0 = ones, row1 = [d_0..d_3] 512-chunks.
            lhsH = [stage.tile([2, H * (N // 2)], f32, name=f"lhs{x}")
                    for x in range(2)]
            rhsH = [stage.tile([2, H * (N // 2)], f32, name=f"rhs{x}")
                    for x in range(2)]
            # diag(g_h[chunk c]) tiles: dp[c][:, h*128:(h+1)*128], bf16
            dps = [stage.tile([128, H * 128], bf16, name=f"dp{c}")
                   for c in range(NC)]
            sdg_sb = stage.tile([128, NT * 12], f32)   # sdg chunks, [i, c*12+k]

            # main-loop pools open before the prologue so the first cnt
            # loads can issue ahead of the (long) staging chain; PSUM tiles
            # allocate lazily, after the prologue psum pool has closed.
            with tc.tile_pool(name="mn", bufs=CFG["mn"]) as mn, \
                 tc.tile_pool(name="lp", bufs=CFG["lp"]) as lp, \
                 tc.tile_pool(name="mp", bufs=CFG["mp"]) as mp, \
                 tc.tile_pool(name="ob", bufs=CFG["ob"]) as obp, \
                 tc.tile_pool(name="cntp", bufs=CFG["cnt"]) as cntp:
                order = list(range(NT)) + ([0] if CFG["redo0"] else [])
                cnt_pref = {}

                def load_cnt(idx):
                    it = order[idx]
                    t = cntp.tile([128, 2 * N], bf16, tag="cnt",
                                  name=f"cnt{idx}")
                    cnap = cn_d[:, :]
                    src_ap = bass.AP(
                        tensor=cnap.tensor, offset=it * 128 * N,
                        ap=[[N, 128], [N * N, 2], [1, N]])
                    nc.sync.dma_start(
                        t.rearrange("p (b j) -> p b j", b=2), src_ap)
                    return t

                # ---- prologue (processed in N/2 halves so half-A's
                # staging is ready long before half-B's is needed) ----
                with tc.tile_pool(name="pro", bufs=1) as pro, \
                     tc.tile_pool(name="ppt", bufs=CFG["ppt"],
                                  space="PSUM") as ppt, \
                     tc.tile_pool(name="pps", bufs=2, space="PSUM") as pps:
                    chunksH = [pro.tile([128, N // 2], f32, name=f"ch{x}")
                               for x in range(2)]
                    # src + first cnt tiles lead the SP/HWDGE queue: they
                    # head the critical path
                    for x in range(2):
                        nc.sync.dma_start(
                            chunksH[x].rearrange("p (c f) -> p c f", c=NT // 2),
                            src_d[x * 512:(x + 1) * 512, :]
                            .rearrange("(c p) f -> p c f", p=128))
                    cnt_pref[0] = load_cnt(0)
                    if len(order) > 1:
                        cnt_pref[1] = load_cnt(1)
                    ones_t = pro.tile([1, N // 2], f32)
                    nc.vector.memset(ones_t, 1.0)
                    oap1 = ones_t[0:1, :]
                    ones_b = bass.AP(tensor=oap1.tensor, offset=oap1.offset,
                                     ap=[oap1.ap[0], [0, H], oap1.ap[1]])
                    for x in range(2):
                        nc.gpsimd.dma_start(lhsH[x][1:2, :], ones_b)
                        nc.gpsimd.dma_start(rhsH[x][0:1, :], ones_b)
                    ident = pro.tile([128, 128], f32)
                    make_identity(nc, ident)
                    ident_bf = pro.tile([128, 128], bf16)
                    nc.vector.tensor_scalar(ident_bf, ident, 1.0, None,
                                            op0=OP.mult)
                    a_sb = pro.tile([F_IN, 12], f32)
                    nc.gpsimd.dma_start(a_sb, a_d[:, :])
                    srcTH = [pro.tile([128, N // 2], f32, name=f"sT{x}")
                             for x in range(2)]
                    sdgTH = [pro.tile([12, N // 2], f32, name=f"dT{x}")
                             for x in range(2)]
                    for x in range(2):
                        cview = chunksH[x].rearrange("p (c f) -> p c f",
                                                     c=NT // 2)
                        for cc in range(NT // 2):
                            pt = ppt.tile([128, 128], f32, tag="pt")
                            nc.tensor.transpose(pt, cview[:, cc, :], ident)
                            if CFG["dve_srct"]:
                                nc.vector.tensor_scalar(
                                    srcTH[x][:, cc * 128:(cc + 1) * 128],
                                    pt, 1.0, None, op0=OP.mult)
                            else:
                                nc.scalar.copy(
                                    srcTH[x][:, cc * 128:(cc + 1) * 128], pt)
                        ps = pps.tile([12, 512], f32, tag="sdg")
                        nc.tensor.matmul(ps, a_sb, srcTH[x],
                                         start=True, stop=True)
                        if CFG["dve_copies"]:
                            nc.vector.tensor_scalar(sdgTH[x], ps, 1.0, None,
                                                    op0=OP.mult)
                        else:
                            nc.scalar.copy(sdgTH[x], ps)
                        # staging rows for this half: s chunks + d chunks
                        nc.sync.dma_start(lhsH[x][0:1, :], sdgTH[x][0:4, :])
                        nc.sync.dma_start(rhsH[x][1:2, :], sdgTH[x][4:8, :])
                        # per-partition g columns + diag tiles for this half
                        for cc in range(NT // 2):
                            c = x * (NT // 2) + cc
                            ps12 = pps.tile([128, 12], f32, tag="sdgc")
                            nc.tensor.matmul(
                                ps12, srcTH[x][:, cc * 128:(cc + 1) * 128],
                                a_sb, start=True, stop=True)
                            if CFG["dve_copies"]:
                                nc.vector.tensor_scalar(
                                    sdg_sb[:, c * 12:(c + 1) * 12], ps12,
                                    1.0, None, op0=OP.mult)
                            else:
                                nc.scalar.copy(
                                    sdg_sb[:, c * 12:(c + 1) * 12], ps12)
                            for h in range(H):
                                g_col = sdg_sb[:, c * 12 + 8 + h:
                                               c * 12 + 9 + h]
                                eng = nc.vector if x == 0 else nc.gpsimd
                                eng.tensor_scalar(
                                    dps[c][:, h * 128:(h + 1) * 128],
                                    ident_bf, g_col, None, op0=OP.mult)

                def lhsP(h, i0):
                    return lhsH[i0 // 512][:, h * 512 + (i0 % 512):
                                           h * 512 + (i0 % 512) + 128]

                def rhsP(h, j0):
                    return rhsH[j0 // 512][:, h * 512:(h + 1) * 512]

                psp_cm = tc.tile_pool(name="ps", bufs=2, space="PSUM")
                psp = psp_cm.__enter__()
                # j-block-major order: the first NT tiles only need rhs
                # half A and diag chunks 0..3; cnt tiles stay resident and
                # are reused by the second pass.
                tiles = [(idx, it, jb) for jb in range(N // JB)
                         for idx, it in enumerate(order)]

                # Software pipeline: stage A at tile t, B at t-1, C at t-2,
                # D at t-3.  Every instruction an engine dequeues has inputs
                # that finished >= 1 tile ago, so the in-order engine queues
                # never head-of-line block on cross-engine round trips.
                state = {}

                def stage_a(t):
                    idx, it, jb = tiles[t]
                    i0, j0 = it * 128, jb * JB
                    if jb == 0:
                        cn_t = cnt_pref.pop(idx)
                        if idx + 2 < len(order):
                            cnt_pref[idx + 2] = load_cnt(idx + 2)
                        state[("cn", idx)] = cn_t
                    cn_t = state[("cn", idx)]
                    cnt_t = cn_t[:, 0:N]
                    nct_t = cn_t[:, N:2 * N]
                    dve_route = ((t % 9) in CFG["dve_prelu"]
                                 and t < CFG["dve_prelu_until"])
                    if CFG["inplace_prelu"]:
                        # one full psa tile; prelu runs in place in PSUM and
                        # exp reads PSUM directly (no l round-trip)
                        psa = psp.tile([128, H * JB], f32, tag="psa")
                        for h in range(H):
                            nc.tensor.matmul(
                                psa[:, h * JB:(h + 1) * JB],
                                dps[it][:, h * 128:(h + 1) * 128],
                                cnt_t[:, j0:j0 + JB],
                                start=True, stop=True)
                        pv = psa.rearrange("p (h j) -> p h j", h=H)
                        for c in range(JB // 128):
                            jc = jb * (JB // 128) + c
                            nc.tensor.matmul(
                                pv[:, :, c * 128:(c + 1) * 128],
                                nct_t[:, jc * 128:(jc + 1) * 128],
                                dps[jc],
                                start=False, stop=True,
                                skip_group_check=True)
                        for h in range(H):
                            nc.tensor.matmul(
                                psa[:, h * JB:(h + 1) * JB],
                                lhsP(h, i0).bitcast(f32r),
                                rhsP(h, j0).bitcast(f32r),
                                start=False, stop=True,
                                skip_group_check=True)
                        if CFG["ablate"] >= 6:
                            return
                        if dve_route:
                            nc.vector._custom_dve(
                                prelu_op, out=psa[:, :], in0=psa,
                                imm2=NEG_SLOPE)
                        else:
                            nc.scalar.activation(psa[:, :], psa, AF.Prelu,
                                                 alpha=NEG_SLOPE)
                        state[("l", t)] = psa
                        return
                    l_t = lp.tile([128, H * JB], f32, tag="l")
                    # two half-psa tiles (heads 0-1 / 2-3, 2 banks each,
                    # 4 psum bufs): PE for tile t+2 only waits on the
                    # matching half's prelu, halving the psa-reuse stall
                    for hp in range(2):
                        psa = psp.tile([128, 2 * JB], f32, tag=f"psa{hp}")
                        for k in range(2):
                            h = hp * 2 + k
                            nc.tensor.matmul(
                                psa[:, k * JB:(k + 1) * JB],
                                dps[it][:, h * 128:(h + 1) * 128],
                                cnt_t[:, j0:j0 + JB],
                                start=True, stop=True)
                        pv = psa.rearrange("p (h j) -> p h j", h=2)
                        for c in range(JB // 128):
                            jc = jb * (JB // 128) + c
                            nc.tensor.matmul(
                                pv[:, :, c * 128:(c + 1) * 128],
                                nct_t[:, jc * 128:(jc + 1) * 128],
                                dps[jc][:, hp * 256:(hp + 1) * 256],
                                start=False, stop=True,
                                skip_group_check=True)
                        for k in range(2):
                            h = hp * 2 + k
                            nc.tensor.matmul(
                                psa[:, k * JB:(k + 1) * JB],
                                lhsP(h, i0).bitcast(f32r),
                                rhsP(h, j0).bitcast(f32r),
                                start=False, stop=True,
                                skip_group_check=True)
                        if CFG["ablate"] >= 6:
                            continue
                        lv = l_t[:, hp * 2 * JB:(hp + 1) * 2 * JB]
                        if dve_route:
                            nc.vector._custom_dve(
                                prelu_op, out=lv, in0=psa, imm2=NEG_SLOPE)
                        else:
                            nc.scalar.activation(lv, psa, AF.Prelu,
                                                 alpha=NEG_SLOPE)
                    if CFG["ablate"] >= 6:
                        return
                    state[("l", t)] = l_t

                def stage_b(t):
                    if CFG["ablate"] >= 5:
                        return
                    l_t = state.pop(("l", t))
                    m_t = mp.tile([128, H * JB], bf16, tag="m")
                    nc.scalar.activation(m_t, l_t, AF.Exp)
                    state[("m", t)] = m_t

                def stage_e(t):
                    if CFG["ablate"] >= 4:
                        return
                    m_t = state[("m", t)]
                    s2 = mn.tile([128, 2 * JB], bf16, tag="s2")
                    s2eng = (nc.gpsimd if ((t % 4) in CFG["pool_s2"]
                             or t >= CFG["ps2_from"]) else nc.vector)
                    s2eng.tensor_tensor(
                        s2, m_t[:, 0:2 * JB], m_t[:, 2 * JB:4 * JB],
                        op=OP.add)
                    if CFG["fuse_rsum"]:
                        state[("s2", t)] = s2
                    else:
                        s_f = mn.tile([128, JB], f32, tag="s")
                        s_eng = (nc.vector if t >= CFG["s_dve_from"]
                                 else nc.gpsimd)
                        s_eng.tensor_tensor(
                            s_f, s2[:, 0:JB], s2[:, JB:2 * JB], op=OP.add)
                        state[("s", t)] = s_f

                def stage_c(t):
                    if CFG["ablate"] >= 4 or CFG["divide"]:
                        return
                    if CFG["fuse_rsum"]:
                        s2 = state.pop(("s2", t))
                        r_b = mn.tile([128, JB], bf16, tag="rb")
                        from concourse.dve_ops import RECIP_APPROX_FAST_CONSTS
                        cc = RECIP_APPROX_FAST_CONSTS
                        nc.vector._custom_dve(
                            rsum_op, out=r_b[:, :], in0=s2[:, 0:JB],
                            in1=s2[:, JB:2 * JB], s0=cc["s0"], s1=cc["s1"])
                        state[("r", t)] = r_b
                        return
                    s_f = state.pop(("s", t))
                    if CFG["ablate"] >= 3:
                        return
                    # custom NR reciprocal computes in the f32 pipeline and
                    # converts to bf16 at the write, saving a convert pass
                    from concourse.dve_ops import (RECIP_APPROX_FAST_CONSTS,
                                                   RECIPROCAL_APPROX_FAST)
                    r_b = mn.tile([128, JB], bf16, tag="rb")
                    cc = RECIP_APPROX_FAST_CONSTS
                    nc.vector._custom_dve(
                        RECIPROCAL_APPROX_FAST, out=r_b[:, :], in0=s_f[:, :],
                        s0=cc["s0"], s1=cc["s1"], imm2=cc["imm2"])
                    state[("r", t)] = r_b

                def stage_d(t):
                    if CFG["ablate"] >= 3:
                        return
                    idx, it, jb = tiles[t]
                    i0, j0 = it * 128, jb * JB
                    m_t = state.pop(("m", t))
                    r_b = state.pop(("r", t))
                    rap = r_b[:, :]
                    r_b4 = bass.AP(tensor=rap.tensor, offset=rap.offset,
                                   ap=[rap.ap[0], [0, H], rap.ap[1]])
                    o_t = obp.tile([128, H * JB], bf16, tag="o")
                    o3 = o_t.rearrange("p (h j) -> p h j", h=H)
                    m3 = m_t.rearrange("p (h j) -> p h j", h=H)
                    if t >= len(tiles) - CFG["split_last"]:
                        # closing tiles: mult+store in plane-halves so the
                        # final DMA overlaps the second half's multiply
                        r_b2 = bass.AP(tensor=rap.tensor, offset=rap.offset,
                                       ap=[rap.ap[0], [0, 2], rap.ap[1]])
                        for g in range(2):
                            nc.vector.tensor_tensor(
                                o3[:, 2 * g:2 * g + 2],
                                m3[:, 2 * g:2 * g + 2], r_b2, op=OP.mult)
                            nc.sync.dma_start(
                                out_d[2 * g:2 * g + 2, i0:i0 + 128,
                                      j0:j0 + JB]
                                .rearrange("h p j -> p h j"),
                                o3[:, 2 * g:2 * g + 2])
                        return
                    ps = CFG["pool_mult"] if t < CFG["pm_cut"] else 0
                    if ps:
                        r_b3 = bass.AP(tensor=rap.tensor, offset=rap.offset,
                                       ap=[rap.ap[0], [0, H - ps], rap.ap[1]])
                        r_b1 = bass.AP(tensor=rap.tensor, offset=rap.offset,
                                       ap=[rap.ap[0], [0, ps], rap.ap[1]])
                        nc.vector.tensor_tensor(
                            o3[:, 0:H - ps], m3[:, 0:H - ps], r_b3, op=OP.mult)
                        nc.gpsimd.tensor_tensor(
                            o3[:, H - ps:H], m3[:, H - ps:H], r_b1, op=OP.mult)
                    else:
                        nc.vector.tensor_tensor(o3, m3, r_b4, op=OP.mult)
                    if CFG["ablate"] >= 1:
                        return
                    nc.sync.dma_start(
                        out_d[:, i0:i0 + 128, j0:j0 + JB]
                        .rearrange("h p j -> p h j"),
                        o_t.rearrange("p (h j) -> p h j", h=H))

                nt_total = len(tiles)
                stage_order = CFG.get("stage_order", "abecd")
                lags = CFG.get("lags", {"a": 0, "b": 1, "e": 2, "c": 3, "d": 4})
                fns = {"a": stage_a, "b": stage_b, "e": stage_e,
                       "c": stage_c, "d": stage_d}
                maxlag = max(lags.values())
                for t in range(nt_total + maxlag):
                    for ch in stage_order:
                        if 0 <= t - lags[ch] < nt_total:
                            fns[ch](t - lags[ch])
                psp_cm.__exit__(None, None, None)
    nc.finalize()
    return nc


def _prepare_in_maps(src, edge_index, W_lin, a_src, a_dst, W_edge, a_edge):
    import ml_dtypes

    src = np.ascontiguousarray(np.asarray(src, dtype=np.float32))
    ei = np.asarray(edge_index).astype(np.int64)
    W_lin = np.asarray(W_lin, dtype=np.float32)
    a_src = np.asarray(a_src, dtype=np.float32)
    a_dst = np.asarray(a_dst, dtype=np.float32)
    W_edge = np.asarray(W_edge, dtype=np.float32)
    a_edge = np.asarray(a_edge, dtype=np.float32)

    # fold weights: A = [W_lin@a_src | W_lin@a_dst | W_edge@a_edge]  [128,12]
    A = np.concatenate(
        [W_lin @ a_src, W_lin @ a_dst, W_edge @ a_edge], axis=1
    ).astype(np.float32)
    # edge multiplicity matrix (shared across batches)
    cnt = np.zeros((N, N), np.float32)
    np.add.at(cnt, (ei[0], ei[1]), 1.0)
    # -cnt^T packed per-i-tile: ncntT[it*128+p, q*128+f] = -cnt[it*128+f, q*128+p]
    T = np.ascontiguousarray((-cnt).T)          # T[j, i] = -cnt[i, j]
    ncntT = T.reshape(NT, 128, NT, 128).transpose(2, 1, 0, 3).reshape(N, N)
    cn = np.ascontiguousarray(
        np.concatenate([cnt, ncntT], axis=0)).astype(ml_dtypes.bfloat16)
    return [
        {"src": np.ascontiguousarray(src[b]), "cn": cn, "A": A}
        for b in range(B)
    ]


def kernel(src, edge_index, W_lin, a_src, a_dst, W_edge, a_edge):
    from concourse.bass_utils import run_bass_kernel_spmd

    in_maps = _prepare_in_maps(src, edge_index, W_lin, a_src, a_dst,
                               W_edge, a_edge)
    nc = _build_nc()
    res = run_bass_kernel_spmd(nc, in_maps, core_ids=list(range(B)))
    out = np.stack(
        [np.asarray(res.results[b]["out"]).astype(np.float32)
         .transpose(1, 2, 0) for b in range(B)], axis=0)
    return np.ascontiguousarray(out)


if __name__ == "__main__":
    rng = np.random.default_rng(0)
    inputs = {
        "src": rng.standard_normal((B, N, F_IN), dtype=np.float32),
        "edge_index": rng.integers(0, N, (2, 32768)).astype(np.int32),
        "W_lin": rng.standard_normal((F_IN, 128), dtype=np.float32) / np.sqrt(F_IN),
        "a_src": rng.standard_normal((128, H), dtype=np.float32) / np.sqrt(128),
        "a_dst": rng.standard_normal((128, H), dtype=np.float32) / np.sqrt(128),
        "W_edge": rng.standard_normal((F_IN, 64), dtype=np.float32) / np.sqrt(F_IN),
        "a_edge": rng.standard_normal((64, H), dtype=np.float32) / np.sqrt(64),
    }
    out = kernel(**inputs)
    print("out", out.shape, out.dtype, out.sum())


# revision 78
# speedup vs baseline: 1.0070x; 1.0070x over previous
"""Trainium2 Bass kernel for nn_AttentionLayer_48722109006175.

Math: out[b,i,j,h] = softmax_h( leaky_relu( s[b,i,h] + d[b,j,h]
                                            + cnt[i,j]*(g[b,i,h]-g[b,j,h]), 0.2 ) )

with s = src@(W_lin@a_src), d = src@(W_lin@a_dst), g = src@(W_edge@a_edge)
and cnt[i,j] the (batch-independent) edge multiplicity matrix.

All three logit contributions accumulate on the PE into one PSUM tile
psa [128, 4*512] (head-blocks of 512 j's), per (i-tile, j-block):
  P:    psa[h] += s_h[i] + d_h[j]          rank-2 f32r matmul (free 512)
  row:  psa[h] += g_h[i]*cnt[i,j]          diag(g_h[i-tile]) @ cnt   (bf16)
  col:  psa[h] -= g_h[j]*cnt[i,j]          (-cnt^T chunk) @ diag-pair (bf16,
        one matmul per 128-j chunk covering all 4 heads via a strided
        PSUM out AP)
All are 1 cycle/row on PE, so the edge scatter costs the same as the
rank-2 part and no dense DVE cnt-multiply pass is needed.

The elementwise tail is software-pipelined across tiles (stages lag
1-4 tiles so no in-order engine queue ever head-of-line blocks on a
cross-engine dependency):
  A(t):   PE accumulate + prelu (a registered single-input custom-DVE
          op max(x, 0.2x) on ~half the tiles, ACT Prelu on the rest)
  B(t-1): ACT exp -> m bf16
  E(t-2): DVE pair-sum (bf16 2x mode)
  C(t-3): custom-DVE fused sum+reciprocal (seed + one Newton step,
          +-0.17% - below the bf16 write quantization), bf16 out
  D(t-4): DVE broadcast multiply (bf16 2x mode, half offloaded to
          Pool except for the last tiles) + store
Output is stored bf16 in head-plane layout [H,N,N]; the host
transposes to [N,N,H] f32.  cnt and -cnt^T (packed per-i-tile) ship
as one concatenated bf16 tensor, one load per i-tile (all 8 stay
resident; the j-block-major tile order reuses them).

Startup latency: the prologue runs in N/2 halves (src load ->
transpose -> sdg -> staging rows), with the src and first cnt DMAs
leading the SP queue and ones/staging broadcasts on the Pool queue,
so the first i-tile pass starts ~11us in, needing only half-A
staging.  psa is split into two 2-bank half-tiles (head pairs) so
PE's psa reuse only waits on the matching half's prelu.  One quarter
of the final multiply runs on the Pool engine (except for the last
tiles, where Pool would straggle the closing store).
Sharding: data-parallel over batch, one batch per NeuronCore.
Measured: 70592 ns (cost-model timeline), absmax rel err 9.6e-3
(vs 122886 ns baseline, 1.74x).
"""

import numpy as np

B, N, F_IN, H = 8, 1024, 128, 4
JB = 512          # j-block
NT = N // 128     # 8 i-tiles
NC = N // 128     # 8 j-chunks of 128
NEG_SLOPE = 0.2


def _leaky_relu_dve_op():
    """Register (once) a single-input custom-DVE op computing
    out = max(x, NEG_SLOPE*x).  A plain scalar_tensor_tensor(psa, c, psa)
    reads PSUM twice, which the DVE forbids; this op reads Src0 once.
    Registration follows the documented extension path in dve_ops.py
    (append to OPS + the name->row map); the per-NEFF uop table is then
    generated by the normal compile_bir_kernel flow."""
    import numpy as np
    import concourse.dve_ops as dve_ops
    from concourse.dve_spec import Spec, Src0, C2, maxx, lower, _has_src1
    from concourse.dve_uop import DveOpSpec

    NAME = "PRELU_LEAKY_ANT"
    for op in dve_ops.OPS:
        if op.name == NAME:
            return op
    spec = Spec(
        body=maxx(Src0, Src0 * C2),
        reference=lambda in0, in1, s0, s1, imm2: np.maximum(
            in0, in0 * imm2).astype(np.float32),
    )
    row = max(dve_ops._SUB_OPCODE_FOR_NAME.values()) + 1
    assert row < 0x20
    shas = {}
    for ver in ("v3", "v4"):
        compiled = DveOpSpec(name=NAME, opcode=row, uops=lower(spec, ver=ver),
                             rd1_en=_has_src1(spec))
        shas[ver] = compiled.sha(ver)
    op = dve_ops.DveOp(NAME, spec, subdim=False, uops_sha=shas)
    dve_ops.OPS.append(op)
    dve_ops._SUB_OPCODE_FOR_NAME[NAME] = row
    dve_ops.CUSTOM_DVE_SPECS[NAME] = spec
    return op


def _recip_sum_dve_op():
    """Register (once) a custom-DVE op: out = approx 1/(Src0+Src1).

    BITWISE_NOT exponent-flip seed + ONE inline Newton pass with the
    RECIP_APPROX_FAST Chebyshev pair (those constants equioscillate the
    post-NR error at ~0.17%, below the bf16 quantization of the result,
    so the second NR pass of the library op buys nothing here).  Fusing
    the final head-pair sum into the reciprocal keeps the whole softmax
    normalize on the DVE - no Pool round trip per tile."""
    import numpy as np
    import concourse.dve_ops as dve_ops
    from concourse.dve_spec import (Spec, Src0, Src1, C0, C1, AluOp, Bin,
                                    lower, _has_src1)
    from concourse.dve_uop import DveOpSpec

    NAME = "RECIP_SUM_ANT"
    for op in dve_ops.OPS:
        if op.name == NAME:
            return op

    S = Src0 + Src1
    nx = Bin(AluOp.BITWISE_NOT, S, S)
    y0 = nx * C0
    body = y0 * (C1 - S * y0)

    def ref(in0, in1, s0, s1, imm2):
        s = (np.asarray(in0, np.float32) + np.asarray(in1, np.float32))
        not_x = (~s.view(np.int32)).view(np.float32)
        y0 = not_x * s0
        return (y0 * (s1 - s * y0)).astype(np.float32)

    spec = Spec(body=body, reference=ref)
    row = max(dve_ops._SUB_OPCODE_FOR_NAME.values()) + 1
    assert row < 0x20
    shas = {}
    for ver in ("v3", "v4"):
        compiled = DveOpSpec(name=NAME, opcode=row, uops=lower(spec, ver=ver),
                             rd1_en=_has_src1(spec))
        shas[ver] = compiled.sha(ver)
    op = dve_ops.DveOp(NAME, spec, subdim=False, uops_sha=shas)
    dve_ops.OPS.append(op)
    dve_ops._SUB_OPCODE_FOR_NAME[NAME] = row
    dve_ops.CUSTOM_DVE_SPECS[NAME] = spec
    return op


CFG = {
    "dve_prelu": (1, 2, 4, 7, 8),  # t%9 residues routed to DVE prelu
    "store_per_tile": False,      # (unused in staged pipeline)
    "lp": 4, "mp": 7, "ob": 4, "mn": 6, "cnt": 8,
    "redo0": False,               # recompute i-tile 0 at the end
    "ablate": 0,                  # 0=full .. 6=PE only (debug)
    "stage_order": "abecd",
    "divide": False,
    "pool_s2": (),
    "pool_mult": 2,
    "pm_cut": 14,
    "s_dve_from": 99,
    "fuse_rsum": True,
    "dve_copies": True,
    "ps2_from": 99,
    "dve_prelu_until": 99,
    "dve_srct": False,
    "split_last": 2,
    "ppt": 2,
    "inplace_prelu": False,
    "half_split_prelu": True,
    "hsp_alt": True,
}


def _build_nc():
    import concourse.bass as bass
    import concourse.bacc as bacc
    import concourse.mybir as mybir
    import concourse.tile as tile
    from concourse.masks import make_identity

    prelu_op = _leaky_relu_dve_op()
    rsum_op = _recip_sum_dve_op()

    f32 = mybir.dt.float32
    f32r = mybir.dt.float32r
    bf16 = mybir.dt.bfloat16
    AF = mybir.ActivationFunctionType
    OP = mybir.AluOpType

    nc = bacc.Bacc()
    # Reset DMA queues + clear bass-managed semaphores at kernel entry.
    # (Bass only emits this when target_bir_lowering=True; without it, stale
    # semaphore/DMA state from previously-executed NEFFs on the same core
    # races the first tile loads.)
    from concourse.bass import compact_to_ranges
    for sem_range in compact_to_ranges(
        [s for s in nc._kernel_sem_range if s not in nc.barrier_sems]
    ):
        nc.gpsimd.dma_reset(sem_range)
        nc.gpsimd.sem_clear(sem_range)
    nc._nrt_pseudo_barrier()

    src_d = nc.dram_tensor("src", [N, F_IN], f32, kind="ExternalInput")
    # rows 0..N: cnt (bf16); rows N..2N: -cnt^T packed per-i-tile slab
    cn_d = nc.dram_tensor("cn", [2 * N, N], bf16, kind="ExternalInput")
    a_d = nc.dram_tensor("A", [F_IN, 12], f32, kind="ExternalInput")
    out_d = nc.dram_tensor("out", [H, N, N], bf16, kind="ExternalOutput")

    with tile.TileContext(nc) as tc:
        with tc.tile_pool(name="stage", bufs=1) as stage:
            # P-matmul staging, split by N/2 halves so the first j-block
            # pass only waits on the half-A prologue chain:
            # lhs row0 = [s_0..s_3] 512-chunks, row1 = ones;
            # rhs row0 = ones, row1 = [d_0..d_3] 512-chunks.
            lhsH = [stage.tile([2, H * (N // 2)], f32, name=f"lhs{x}")
                    for x in range(2)]
            rhsH = [stage.tile([2, H * (N // 2)], f32, name=f"rhs{x}")
                    for x in range(2)]
            # diag(g_h[chunk c]) tiles: dp[c][:, h*128:(h+1)*128], bf16
            dps = [stage.tile([128, H * 128], bf16, name=f"dp{c}")
                   for c in range(NC)]
            sdg_sb = stage.tile([128, NT * 12], f32)   # sdg chunks, [i, c*12+k]

            # main-loop pools open before the prologue so the first cnt
            # loads can issue ahead of the (long) staging chain; PSUM tiles
            # allocate lazily, after the prologue psum pool has closed.
            with tc.tile_pool(name="mn", bufs=CFG["mn"]) as mn, \
                 tc.tile_pool(name="lp", bufs=CFG["lp"]) as lp, \
                 tc.tile_pool(name="mp", bufs=CFG["mp"]) as mp, \
                 tc.tile_pool(name="ob", bufs=CFG["ob"]) as obp, \
                 tc.tile_pool(name="cntp", bufs=CFG["cnt"]) as cntp:
                order = list(range(NT)) + ([0] if CFG["redo0"] else [])
                cnt_pref = {}

                def load_cnt(idx):
                    it = order[idx]
                    t = cntp.tile([128, 2 * N], bf16, tag="cnt",
                                  name=f"cnt{idx}")
                    cnap = cn_d[:, :]
                    src_ap = bass.AP(
                        tensor=cnap.tensor, offset=it * 128 * N,
                        ap=[[N, 128], [N * N, 2], [1, N]])
                    nc.sync.dma_start(
                        t.rearrange("p (b j) -> p b j", b=2), src_ap)
                    return t

                # ---- prologue (processed in N/2 halves so half-A's
                # staging is ready long before half-B's is needed) ----
                with tc.tile_pool(name="pro", bufs=1) as pro, \
                     tc.tile_pool(name="ppt", bufs=CFG["ppt"],
                                  space="PSUM") as ppt, \
                     tc.tile_pool(name="pps", bufs=2, space="PSUM") as pps:
                    chunksH = [pro.tile([128, N // 2], f32, name=f"ch{x}")
                               for x in range(2)]
                    # src + first cnt tiles lead the SP/HWDGE queue: they
                    # head the critical path
                    for x in range(2):
                        nc.sync.dma_start(
                            chunksH[x].rearrange("p (c f) -> p c f", c=NT // 2),
                            src_d[x * 512:(x + 1) * 512, :]
                            .rearrange("(c p) f -> p c f", p=128))
                    cnt_pref[0] = load_cnt(0)
                    if len(order) > 1:
                        cnt_pref[1] = load_cnt(1)
                    ones_t = pro.tile([1, N // 2], f32)
                    nc.vector.memset(ones_t, 1.0)
                    oap1 = ones_t[0:1, :]
                    ones_b = bass.AP(tensor=oap1.tensor, offset=oap1.offset,
                                     ap=[oap1.ap[0], [0, H], oap1.ap[1]])
                    for x in range(2):
                        nc.gpsimd.dma_start(lhsH[x][1:2, :], ones_b)
                        nc.gpsimd.dma_start(rhsH[x][0:1, :], ones_b)
                    ident = pro.tile([128, 128], f32)
                    make_identity(nc, ident)
                    ident_bf = pro.tile([128, 128], bf16)
                    nc.vector.tensor_scalar(ident_bf, ident, 1.0, None,
                                            op0=OP.mult)
                    a_sb = pro.tile([F_IN, 12], f32)
                    nc.gpsimd.dma_start(a_sb, a_d[:, :])
                    srcTH = [pro.tile([128, N // 2], f32, name=f"sT{x}")
                             for x in range(2)]
                    sdgTH = [pro.tile([12, N // 2], f32, name=f"dT{x}")
                             for x in range(2)]
                    for x in range(2):
                        cview = chunksH[x].rearrange("p (c f) -> p c f",
                                                     c=NT // 2)
                        for cc in range(NT // 2):
                            pt = ppt.tile([128, 128], f32, tag="pt")
                            nc.tensor.transpose(pt, cview[:, cc, :], ident)
                            if CFG["dve_srct"]:
                                nc.vector.tensor_scalar(
                                    srcTH[x][:, cc * 128:(cc + 1) * 128],
                                    pt, 1.0, None, op0=OP.mult)
                            else:
                                nc.scalar.copy(
                                    srcTH[x][:, cc * 128:(cc + 1) * 128], pt)
                        ps = pps.tile([12, 512], f32, tag="sdg")
                        nc.tensor.matmul(ps, a_sb, srcTH[x],
                                         start=True, stop=True)
                        if CFG["dve_copies"]:
                            nc.vector.tensor_scalar(sdgTH[x], ps, 1.0, None,
                                                    op0=OP.mult)
                        else:
                            nc.scalar.copy(sdgTH[x], ps)
                        # staging rows for this half: s chunks + d chunks
                        nc.sync.dma_start(lhsH[x][0:1, :], sdgTH[x][0:4, :])
                        nc.sync.dma_start(rhsH[x][1:2, :], sdgTH[x][4:8, :])
                        # per-partition g columns + diag tiles for this half
                        for cc in range(NT // 2):
                            c = x * (NT // 2) + cc
                            ps12 = pps.tile([128, 12], f32, tag="sdgc")
                            nc.tensor.matmul(
                                ps12, srcTH[x][:, cc * 128:(cc + 1) * 128],
                                a_sb, start=True, stop=True)
                            if CFG["dve_copies"]:
                                nc.vector.tensor_scalar(
                                    sdg_sb[:, c * 12:(c + 1) * 12], ps12,
                                    1.0, None, op0=OP.mult)
                            else:
                                nc.scalar.copy(
                                    sdg_sb[:, c * 12:(c + 1) * 12], ps12)
                            for h in range(H):
                                g_col = sdg_sb[:, c * 12 + 8 + h:
                                               c * 12 + 9 + h]
                                eng = nc.vector if x == 0 else nc.gpsimd
                                eng.tensor_scalar(
                                    dps[c][:, h * 128:(h + 1) * 128],
                                    ident_bf, g_col, None, op0=OP.mult)

                def lhsP(h, i0):
                    return lhsH[i0 // 512][:, h * 512 + (i0 % 512):
                                           h * 512 + (i0 % 512) + 128]

                def rhsP(h, j0):
                    return rhsH[j0 // 512][:, h * 512:(h + 1) * 512]

                psp_cm = tc.tile_pool(name="ps", bufs=2, space="PSUM")
                psp = psp_cm.__enter__()
                # j-block-major order: the first NT tiles only need rhs
                # half A and diag chunks 0..3; cnt tiles stay resident and
                # are reused by the second pass.
                tiles = [(idx, it, jb) for jb in range(N // JB)
                         for idx, it in enumerate(order)]

                # Software pipeline: stage A at tile t, B at t-1, C at t-2,
                # D at t-3.  Every instruction an engine dequeues has inputs
                # that finished >= 1 tile ago, so the in-order engine queues
                # never head-of-line block on cross-engine round trips.
                state = {}

                def stage_a(t):
                    idx, it, jb = tiles[t]
                    i0, j0 = it * 128, jb * JB
                    if jb == 0:
                        cn_t = cnt_pref.pop(idx)
                        if idx + 2 < len(order):
                            cnt_pref[idx + 2] = load_cnt(idx + 2)
                        state[("cn", idx)] = cn_t
                    cn_t = state[("cn", idx)]
                    cnt_t = cn_t[:, 0:N]
                    nct_t = cn_t[:, N:2 * N]
                    dve_route = ((t % 9) in CFG["dve_prelu"]
                                 and t < CFG["dve_prelu_until"])
                    if CFG["inplace_prelu"]:
                        # one full psa tile; prelu runs in place in PSUM and
                        # exp reads PSUM directly (no l round-trip)
                        psa = psp.tile([128, H * JB], f32, tag="psa")
                        for h in range(H):
                            nc.tensor.matmul(
                                psa[:, h * JB:(h + 1) * JB],
                                dps[it][:, h * 128:(h + 1) * 128],
                                cnt_t[:, j0:j0 + JB],
                                start=True, stop=True)
                        pv = psa.rearrange("p (h j) -> p h j", h=H)
                        for c in range(JB // 128):
                            jc = jb * (JB // 128) + c
                            nc.tensor.matmul(
                                pv[:, :, c * 128:(c + 1) * 128],
                                nct_t[:, jc * 128:(jc + 1) * 128],
                                dps[jc],
                                start=False, stop=True,
                                skip_group_check=True)
                        for h in range(H):
                            nc.tensor.matmul(
                                psa[:, h * JB:(h + 1) * JB],
                                lhsP(h, i0).bitcast(f32r),
                                rhsP(h, j0).bitcast(f32r),
                                start=False, stop=True,
                                skip_group_check=True)
                        if CFG["ablate"] >= 6:
                            return
                        if dve_route:
                            nc.vector._custom_dve(
                                prelu_op, out=psa[:, :], in0=psa,
                                imm2=NEG_SLOPE)
                        else:
                            nc.scalar.activation(psa[:, :], psa, AF.Prelu,
                                                 alpha=NEG_SLOPE)
                        state[("l", t)] = psa
                        return
                    l_t = lp.tile([128, H * JB], f32, tag="l")
                    # two half-psa tiles (heads 0-1 / 2-3, 2 banks each,
                    # 4 psum bufs): PE for tile t+2 only waits on the
                    # matching half's prelu, halving the psa-reuse stall
                    for hp in range(2):
                        psa = psp.tile([128, 2 * JB], f32, tag=f"psa{hp}")
                        for k in range(2):
                            h = hp * 2 + k
                            nc.tensor.matmul(
                                psa[:, k * JB:(k + 1) * JB],
                                dps[it][:, h * 128:(h + 1) * 128],
                                cnt_t[:, j0:j0 + JB],
                                start=True, stop=True)
                        pv = psa.rearrange("p (h j) -> p h j", h=2)
                        for c in range(JB // 128):
                            jc = jb * (JB // 128) + c
                            nc.tensor.matmul(
                                pv[:, :, c * 128:(c + 1) * 128],
                                nct_t[:, jc * 128:(jc + 1) * 128],
                                dps[jc][:, hp * 256:(hp + 1) * 256],
                                start=False, stop=True,
                                skip_group_check=True)
                        for k in range(2):
                            h = hp * 2 + k
                            nc.tensor.matmul(
                                psa[:, k * JB:(k + 1) * JB],
                                lhsP(h, i0).bitcast(f32r),
                                rhsP(h, j0).bitcast(f32r),
                                start=False, stop=True,
                                skip_group_check=True)
                        if CFG["ablate"] >= 6:
                            continue
                        lv = l_t[:, hp * 2 * JB:(hp + 1) * 2 * JB]
                        if CFG["half_split_prelu"]:
                            on_dve = (hp + t) % 2 == 0 if \
                                CFG["hsp_alt"] else hp == 1
                        else:
                            on_dve = dve_route
                        if on_dve:
                            nc.vector._custom_dve(
                                prelu_op, out=lv, in0=psa, imm2=NEG_SLOPE)
                        else:
                            nc.scalar.activation(lv, psa, AF.Prelu,
                                                 alpha=NEG_SLOPE)
                    if CFG["ablate"] >= 6:
                        return
                    state[("l", t)] = l_t

                def stage_b(t):
                    if CFG["ablate"] >= 5:
                        return
                    l_t = state.pop(("l", t))
                    m_t = mp.tile([128, H * JB], bf16, tag="m")
                    nc.scalar.activation(m_t, l_t, AF.Exp)
                    state[("m", t)] = m_t

                def stage_e(t):
                    if CFG["ablate"] >= 4:
                        return
                    m_t = state[("m", t)]
                    s2 = mn.tile([128, 2 * JB], bf16, tag="s2")
                    s2eng = (nc.gpsimd if ((t % 4) in CFG["pool_s2"]
                             or t >= CFG["ps2_from"]) else nc.vector)
                    s2eng.tensor_tensor(
                        s2, m_t[:, 0:2 * JB], m_t[:, 2 * JB:4 * JB],
                        op=OP.add)
                    if CFG["fuse_rsum"]:
                        state[("s2", t)] = s2
                    else:
                        s_f = mn.tile([128, JB], f32, tag="s")
                        s_eng = (nc.vector if t >= CFG["s_dve_from"]
                                 else nc.gpsimd)
                        s_eng.tensor_tensor(
                            s_f, s2[:, 0:JB], s2[:, JB:2 * JB], op=OP.add)
                        state[("s", t)] = s_f

                def stage_c(t):
                    if CFG["ablate"] >= 4 or CFG["divide"]:
                        return
                    if CFG["fuse_rsum"]:
                        s2 = state.pop(("s2", t))
                        r_b = mn.tile([128, JB], bf16, tag="rb")
                        from concourse.dve_ops import RECIP_APPROX_FAST_CONSTS
                        cc = RECIP_APPROX_FAST_CONSTS
                        nc.vector._custom_dve(
                            rsum_op, out=r_b[:, :], in0=s2[:, 0:JB],
                            in1=s2[:, JB:2 * JB], s0=cc["s0"], s1=cc["s1"])
                        state[("r", t)] = r_b
                        return
                    s_f = state.pop(("s", t))
                    if CFG["ablate"] >= 3:
                        return
                    # custom NR reciprocal computes in the f32 pipeline and
                    # converts to bf16 at the write, saving a convert pass
                    from concourse.dve_ops import (RECIP_APPROX_FAST_CONSTS,
                                                   RECIPROCAL_APPROX_FAST)
                    r_b = mn.tile([128, JB], bf16, tag="rb")
                    cc = RECIP_APPROX_FAST_CONSTS
                    nc.vector._custom_dve(
                        RECIPROCAL_APPROX_FAST, out=r_b[:, :], in0=s_f[:, :],
                        s0=cc["s0"], s1=cc["s1"], imm2=cc["imm2"])
                    state[("r", t)] = r_b

                def stage_d(t):
                    if CFG["ablate"] >= 3:
                        return
                    idx, it, jb = tiles[t]
                    i0, j0 = it * 128, jb * JB
                    m_t = state.pop(("m", t))
                    r_b = state.pop(("r", t))
                    rap = r_b[:, :]
                    r_b4 = bass.AP(tensor=rap.tensor, offset=rap.offset,
                                   ap=[rap.ap[0], [0, H], rap.ap[1]])
                    o_t = obp.tile([128, H * JB], bf16, tag="o")
                    o3 = o_t.rearrange("p (h j) -> p h j", h=H)
                    m3 = m_t.rearrange("p (h j) -> p h j", h=H)
                    if t >= len(tiles) - CFG["split_last"]:
                        # closing tiles: mult+store in plane-halves so the
                        # final DMA overlaps the second half's multiply
                        r_b2 = bass.AP(tensor=rap.tensor, offset=rap.offset,
                                       ap=[rap.ap[0], [0, 2], rap.ap[1]])
                        for g in range(2):
                            nc.vector.tensor_tensor(
                                o3[:, 2 * g:2 * g + 2],
                                m3[:, 2 * g:2 * g + 2], r_b2, op=OP.mult)
                            nc.sync.dma_start(
                                out_d[2 * g:2 * g + 2, i0:i0 + 128,
                                      j0:j0 + JB]
                                .rearrange("h p j -> p h j"),
                                o3[:, 2 * g:2 * g + 2])
                        return
                    ps = CFG["pool_mult"] if t < CFG["pm_cut"] else 0
                    if ps:
                        r_b3 = bass.AP(tensor=rap.tensor, offset=rap.offset,
                                       ap=[rap.ap[0], [0, H - ps], rap.ap[1]])
                        r_b1 = bass.AP(tensor=rap.tensor, offset=rap.offset,
                                       ap=[rap.ap[0], [0, ps], rap.ap[1]])
                        nc.vector.tensor_tensor(
                            o3[:, 0:H - ps], m3[:, 0:H - ps], r_b3, op=OP.mult)
                        nc.gpsimd.tensor_tensor(
                            o3[:, H - ps:H], m3[:, H - ps:H], r_b1, op=OP.mult)
                    else:
                        nc.vector.tensor_tensor(o3, m3, r_b4, op=OP.mult)
                    if CFG["ablate"] >= 1:
                        return
                    nc.sync.dma_start(
                        out_d[:, i0:i0 + 128, j0:j0 + JB]
                        .rearrange("h p j -> p h j"),
                        o_t.rearrange("p (h j) -> p h j", h=H))

                nt_total = len(tiles)
                stage_order = CFG.get("stage_order", "abecd")
                lags = CFG.get("lags", {"a": 0, "b": 1, "e": 2, "c": 3, "d": 4})
                fns = {"a": stage_a, "b": stage_b, "e": stage_e,
                       "c": stage_c, "d": stage_d}
                maxlag = max(lags.values())
                for t in range(nt_total + maxlag):
                    for ch in stage_order:
                        if 0 <= t - lags[ch] < nt_total:
                            fns[ch](t - lags[ch])
                psp_cm.__exit__(None, None, None)
    nc.finalize()
    return nc


def _prepare_in_maps(src, edge_index, W_lin, a_src, a_dst, W_edge, a_edge):
    import ml_dtypes

    src = np.ascontiguousarray(np.asarray(src, dtype=np.float32))
    ei = np.asarray(edge_index).astype(np.int64)
    W_lin = np.asarray(W_lin, dtype=np.float32)
    a_src = np.asarray(a_src, dtype=np.float32)
    a_dst = np.asarray(a_dst, dtype=np.float32)
    W_edge = np.asarray(W_edge, dtype=np.float32)
    a_edge = np.asarray(a_edge, dtype=np.float32)

    # fold weights: A = [W_lin@a_src | W_lin@a_dst | W_edge@a_edge]  [128,12]
    A = np.concatenate(
        [W_lin @ a_src, W_lin @ a_dst, W_edge @ a_edge], axis=1
    ).astype(np.float32)
    # edge multiplicity matrix (shared across batches)
    cnt = np.zeros((N, N), np.float32)
    np.add.at(cnt, (ei[0], ei[1]), 1.0)
    # -cnt^T packed per-i-tile: ncntT[it*128+p, q*128+f] = -cnt[it*128+f, q*128+p]
    T = np.ascontiguousarray((-cnt).T)          # T[j, i] = -cnt[i, j]
    ncntT = T.reshape(NT, 128, NT, 128).transpose(2, 1, 0, 3).reshape(N, N)
    cn = np.ascontiguousarray(
        np.concatenate([cnt, ncntT], axis=0)).astype(ml_dtypes.bfloat16)
    return [
        {"src": np.ascontiguousarray(src[b]), "cn": cn, "A": A}
        for b in range(B)
    ]


def kernel(src, edge_index, W_lin, a_src, a_dst, W_edge, a_edge):
    from concourse.bass_utils import run_bass_kernel_spmd

    in_maps = _prepare_in_maps(src, edge_index, W_lin, a_src, a_dst,
                               W_edge, a_edge)
    nc = _build_nc()
    res = run_bass_kernel_spmd(nc, in_maps, core_ids=list(range(B)))
    out = np.stack(
        [np.asarray(res.results[b]["out"]).astype(np.float32)
         .transpose(1, 2, 0) for b in range(B)], axis=0)
    return np.ascontiguousarray(out)


if __name__ == "__main__":
    rng = np.random.default_rng(0)
    inputs = {
        "src": rng.standard_normal((B, N, F_IN), dtype=np.float32),
        "edge_index": rng.integers(0, N, (2, 32768)).astype(np.int32),
        "W_lin": rng.standard_normal((F_IN, 128), dtype=np.float32) / np.sqrt(F_IN),
        "a_src": rng.standard_normal((128, H), dtype=np.float32) / np.sqrt(128),
        "a_dst": rng.standard_normal((128, H), dtype=np.float32) / np.sqrt(128),
        "W_edge": rng.standard_normal((F_IN, 64), dtype=np.float32) / np.sqrt(F_IN),
        "a_edge": rng.standard_normal((64, H), dtype=np.float32) / np.sqrt(64),
    }
    out = kernel(**inputs)
    print("out", out.shape, out.dtype, out.sum())


# revision 86
# speedup vs baseline: 1.0078x; 1.0008x over previous
"""Trainium2 Bass kernel for nn_AttentionLayer_48722109006175.

Math: out[b,i,j,h] = softmax_h( leaky_relu( s[b,i,h] + d[b,j,h]
                                            + cnt[i,j]*(g[b,i,h]-g[b,j,h]), 0.2 ) )

with s = src@(W_lin@a_src), d = src@(W_lin@a_dst), g = src@(W_edge@a_edge)
and cnt[i,j] the (batch-independent) edge multiplicity matrix.

All three logit contributions accumulate on the PE into one PSUM tile
psa [128, 4*512] (head-blocks of 512 j's), per (i-tile, j-block):
  P:    psa[h] += s_h[i] + d_h[j]          rank-2 f32r matmul (free 512)
  row:  psa[h] += g_h[i]*cnt[i,j]          diag(g_h[i-tile]) @ cnt   (bf16)
  col:  psa[h] -= g_h[j]*cnt[i,j]          (-cnt^T chunk) @ diag-pair (bf16,
        one matmul per 128-j chunk covering all 4 heads via a strided
        PSUM out AP)
All are 1 cycle/row on PE, so the edge scatter costs the same as the
rank-2 part and no dense DVE cnt-multiply pass is needed.

The elementwise tail is software-pipelined across tiles (stages lag
1-4 tiles so no in-order engine queue ever head-of-line blocks on a
cross-engine dependency):
  A(t):   PE accumulate + prelu, split within each tile: one psa
          half on ACT Prelu, the other on a registered single-input
          custom-DVE op max(x, 0.2x), alternating halves per tile
  B(t-1): ACT exp -> m bf16
  E(t-2): DVE pair-sum (bf16 2x mode)
  C(t-3): custom-DVE fused sum+reciprocal (seed + one Newton step,
          +-0.17% - below the bf16 write quantization), bf16 out
  D(t-4): DVE broadcast multiply (bf16 2x mode, half offloaded to
          Pool except for the last tiles) + store
Output is stored bf16 in head-plane layout [H,N,N]; the host
transposes to [N,N,H] f32.  cnt and -cnt^T (packed per-i-tile) ship
as one concatenated bf16 tensor, one load per i-tile (all 8 stay
resident; the j-block-major tile order reuses them).

Startup latency: the prologue runs in N/2 halves (src load ->
transpose -> sdg -> staging rows), with the src and first cnt DMAs
leading the SP queue and ones/staging broadcasts on the Pool queue,
so the first i-tile pass starts ~11us in, needing only half-A
staging.  psa is split into two 2-bank half-tiles (head pairs) so
PE's psa reuse only waits on the matching half's prelu.  One quarter
of the final multiply runs on the Pool engine (except for the last
tiles, where Pool would straggle the closing store).
Sharding: data-parallel over batch, one batch per NeuronCore.
Measured: 70049 ns (cost-model timeline), absmax rel err 9.6e-3
(vs 122886 ns baseline, 1.75x).
"""

import numpy as np

B, N, F_IN, H = 8, 1024, 128, 4
JB = 512          # j-block
NT = N // 128     # 8 i-tiles
NC = N // 128     # 8 j-chunks of 128
NEG_SLOPE = 0.2


def _leaky_relu_dve_op():
    """Register (once) a single-input custom-DVE op computing
    out = max(x, NEG_SLOPE*x).  A plain scalar_tensor_tensor(psa, c, psa)
    reads PSUM twice, which the DVE forbids; this op reads Src0 once.
    Registration follows the documented extension path in dve_ops.py
    (append to OPS + the name->row map); the per-NEFF uop table is then
    generated by the normal compile_bir_kernel flow."""
    import numpy as np
    import concourse.dve_ops as dve_ops
    from concourse.dve_spec import Spec, Src0, C2, maxx, lower, _has_src1
    from concourse.dve_uop import DveOpSpec

    NAME = "PRELU_LEAKY_ANT"
    for op in dve_ops.OPS:
        if op.name == NAME:
            return op
    spec = Spec(
        body=maxx(Src0, Src0 * C2),
        reference=lambda in0, in1, s0, s1, imm2: np.maximum(
            in0, in0 * imm2).astype(np.float32),
    )
    row = max(dve_ops._SUB_OPCODE_FOR_NAME.values()) + 1
    assert row < 0x20
    shas = {}
    for ver in ("v3", "v4"):
        compiled = DveOpSpec(name=NAME, opcode=row, uops=lower(spec, ver=ver),
                             rd1_en=_has_src1(spec))
        shas[ver] = compiled.sha(ver)
    op = dve_ops.DveOp(NAME, spec, subdim=False, uops_sha=shas)
    dve_ops.OPS.append(op)
    dve_ops._SUB_OPCODE_FOR_NAME[NAME] = row
    dve_ops.CUSTOM_DVE_SPECS[NAME] = spec
    return op


def _recip_sum_dve_op():
    """Register (once) a custom-DVE op: out = approx 1/(Src0+Src1).

    BITWISE_NOT exponent-flip seed + ONE inline Newton pass with the
    RECIP_APPROX_FAST Chebyshev pair (those constants equioscillate the
    post-NR error at ~0.17%, below the bf16 quantization of the result,
    so the second NR pass of the library op buys nothing here).  Fusing
    the final head-pair sum into the reciprocal keeps the whole softmax
    normalize on the DVE - no Pool round trip per tile."""
    import numpy as np
    import concourse.dve_ops as dve_ops
    from concourse.dve_spec import (Spec, Src0, Src1, C0, C1, AluOp, Bin,
                                    lower, _has_src1)
    from concourse.dve_uop import DveOpSpec

    NAME = "RECIP_SUM_ANT"
    for op in dve_ops.OPS:
        if op.name == NAME:
            return op

    S = Src0 + Src1
    nx = Bin(AluOp.BITWISE_NOT, S, S)
    y0 = nx * C0
    body = y0 * (C1 - S * y0)

    def ref(in0, in1, s0, s1, imm2):
        s = (np.asarray(in0, np.float32) + np.asarray(in1, np.float32))
        not_x = (~s.view(np.int32)).view(np.float32)
        y0 = not_x * s0
        return (y0 * (s1 - s * y0)).astype(np.float32)

    spec = Spec(body=body, reference=ref)
    row = max(dve_ops._SUB_OPCODE_FOR_NAME.values()) + 1
    assert row < 0x20
    shas = {}
    for ver in ("v3", "v4"):
        compiled = DveOpSpec(name=NAME, opcode=row, uops=lower(spec, ver=ver),
                             rd1_en=_has_src1(spec))
        shas[ver] = compiled.sha(ver)
    op = dve_ops.DveOp(NAME, spec, subdim=False, uops_sha=shas)
    dve_ops.OPS.append(op)
    dve_ops._SUB_OPCODE_FOR_NAME[NAME] = row
    dve_ops.CUSTOM_DVE_SPECS[NAME] = spec
    return op


CFG = {
    "dve_prelu": (1, 2, 4, 7, 8),  # t%9 residues routed to DVE prelu
    "store_per_tile": False,      # (unused in staged pipeline)
    "lp": 4, "mp": 7, "ob": 4, "mn": 6, "cnt": 8,
    "redo0": False,               # recompute i-tile 0 at the end
    "ablate": 0,                  # 0=full .. 6=PE only (debug)
    "stage_order": "abecd",
    "divide": False,
    "pool_s2": (),
    "pool_mult": 2,
    "pm_cut": 14,
    "s_dve_from": 99,
    "fuse_rsum": True,
    "dve_copies": True,
    "ps2_from": 99,
    "dve_prelu_until": 99,
    "dve_srct": False,
    "split_last": 2,
    "ppt": 2,
    "inplace_prelu": False,
    "half_split_prelu": True,
    "hsp_alt": True,
    "pdiag_tiles": 0,
    "quad_last": False,
    "both_dve": (),
}


def _build_nc():
    import concourse.bass as bass
    import concourse.bacc as bacc
    import concourse.mybir as mybir
    import concourse.tile as tile
    from concourse.masks import make_identity

    prelu_op = _leaky_relu_dve_op()
    rsum_op = _recip_sum_dve_op()

    f32 = mybir.dt.float32
    f32r = mybir.dt.float32r
    bf16 = mybir.dt.bfloat16
    AF = mybir.ActivationFunctionType
    OP = mybir.AluOpType

    nc = bacc.Bacc()
    # Reset DMA queues + clear bass-managed semaphores at kernel entry.
    # (Bass only emits this when target_bir_lowering=True; without it, stale
    # semaphore/DMA state from previously-executed NEFFs on the same core
    # races the first tile loads.)
    from concourse.bass import compact_to_ranges
    for sem_range in compact_to_ranges(
        [s for s in nc._kernel_sem_range if s not in nc.barrier_sems]
    ):
        nc.gpsimd.dma_reset(sem_range)
        nc.gpsimd.sem_clear(sem_range)
    nc._nrt_pseudo_barrier()

    src_d = nc.dram_tensor("src", [N, F_IN], f32, kind="ExternalInput")
    # rows 0..N: cnt (bf16); rows N..2N: -cnt^T packed per-i-tile slab
    cn_d = nc.dram_tensor("cn", [2 * N, N], bf16, kind="ExternalInput")
    a_d = nc.dram_tensor("A", [F_IN, 12], f32, kind="ExternalInput")
    out_d = nc.dram_tensor("out", [H, N, N], bf16, kind="ExternalOutput")

    with tile.TileContext(nc) as tc:
        with tc.tile_pool(name="stage", bufs=1) as stage:
            # P-matmul staging, split by N/2 halves so the first j-block
            # pass only waits on the half-A prologue chain:
            # lhs row0 = [s_0..s_3] 512-chunks, row1 = ones;
            # rhs row0 = ones, row1 = [d_0..d_3] 512-chunks.
            lhsH = [stage.tile([2, H * (N // 2)], f32, name=f"lhs{x}")
                    for x in range(2)]
            rhsH = [stage.tile([2, H * (N // 2)], f32, name=f"rhs{x}")
                    for x in range(2)]
            # diag(g_h[chunk c]) tiles: dp[c][:, h*128:(h+1)*128], bf16
            dps = [stage.tile([128, H * 128], bf16, name=f"dp{c}")
                   for c in range(NC)]
            sdg_sb = stage.tile([128, NT * 12], f32)   # sdg chunks, [i, c*12+k]

            # main-loop pools open before the prologue so the first cnt
            # loads can issue ahead of the (long) staging chain; PSUM tiles
            # allocate lazily, after the prologue psum pool has closed.
            with tc.tile_pool(name="mn", bufs=CFG["mn"]) as mn, \
                 tc.tile_pool(name="lp", bufs=CFG["lp"]) as lp, \
                 tc.tile_pool(name="mp", bufs=CFG["mp"]) as mp, \
                 tc.tile_pool(name="ob", bufs=CFG["ob"]) as obp, \
                 tc.tile_pool(name="cntp", bufs=CFG["cnt"]) as cntp:
                order = list(range(NT)) + ([0] if CFG["redo0"] else [])
                cnt_pref = {}

                def load_cnt(idx):
                    it = order[idx]
                    t = cntp.tile([128, 2 * N], bf16, tag="cnt",
                                  name=f"cnt{idx}")
                    cnap = cn_d[:, :]
                    src_ap = bass.AP(
                        tensor=cnap.tensor, offset=it * 128 * N,
                        ap=[[N, 128], [N * N, 2], [1, N]])
                    nc.sync.dma_start(
                        t.rearrange("p (b j) -> p b j", b=2), src_ap)
                    return t

                # ---- prologue (processed in N/2 halves so half-A's
                # staging is ready long before half-B's is needed) ----
                with tc.tile_pool(name="pro", bufs=1) as pro, \
                     tc.tile_pool(name="ppt", bufs=CFG["ppt"],
                                  space="PSUM") as ppt, \
                     tc.tile_pool(name="pps", bufs=2, space="PSUM") as pps:
                    chunksH = [pro.tile([128, N // 2], f32, name=f"ch{x}")
                               for x in range(2)]
                    # src + first cnt tiles lead the SP/HWDGE queue: they
                    # head the critical path
                    for x in range(2):
                        nc.sync.dma_start(
                            chunksH[x].rearrange("p (c f) -> p c f", c=NT // 2),
                            src_d[x * 512:(x + 1) * 512, :]
                            .rearrange("(c p) f -> p c f", p=128))
                    cnt_pref[0] = load_cnt(0)
                    if len(order) > 1:
                        cnt_pref[1] = load_cnt(1)
                    ones_t = pro.tile([1, N // 2], f32)
                    nc.vector.memset(ones_t, 1.0)
                    oap1 = ones_t[0:1, :]
                    ones_b = bass.AP(tensor=oap1.tensor, offset=oap1.offset,
                                     ap=[oap1.ap[0], [0, H], oap1.ap[1]])
                    for x in range(2):
                        nc.gpsimd.dma_start(lhsH[x][1:2, :], ones_b)
                        nc.gpsimd.dma_start(rhsH[x][0:1, :], ones_b)
                    ident = pro.tile([128, 128], f32)
                    make_identity(nc, ident)
                    if CFG["pdiag_tiles"]:
                        ones_bf = pro.tile([128, JB], bf16)
                        nc.gpsimd.memset(ones_bf, 1.0)
                        dss = [pro.tile([128, H * 128], bf16, name=f"dss{i}")
                               for i in range(2)]
                        ddp = [pro.tile([128, H * 128], bf16, name=f"ddp{c}")
                               for c in range(4)]
                    ident_bf = pro.tile([128, 128], bf16)
                    nc.vector.tensor_scalar(ident_bf, ident, 1.0, None,
                                            op0=OP.mult)
                    a_sb = pro.tile([F_IN, 12], f32)
                    nc.gpsimd.dma_start(a_sb, a_d[:, :])
                    srcTH = [pro.tile([128, N // 2], f32, name=f"sT{x}")
                             for x in range(2)]
                    sdgTH = [pro.tile([12, N // 2], f32, name=f"dT{x}")
                             for x in range(2)]
                    for x in range(2):
                        cview = chunksH[x].rearrange("p (c f) -> p c f",
                                                     c=NT // 2)
                        for cc in range(NT // 2):
                            pt = ppt.tile([128, 128], f32, tag="pt")
                            nc.tensor.transpose(pt, cview[:, cc, :], ident)
                            if CFG["dve_srct"]:
                                nc.vector.tensor_scalar(
                                    srcTH[x][:, cc * 128:(cc + 1) * 128],
                                    pt, 1.0, None, op0=OP.mult)
                            else:
                                nc.scalar.copy(
                                    srcTH[x][:, cc * 128:(cc + 1) * 128], pt)
                        ps = pps.tile([12, 512], f32, tag="sdg")
                        nc.tensor.matmul(ps, a_sb, srcTH[x],
                                         start=True, stop=True)
                        if CFG["dve_copies"]:
                            nc.vector.tensor_scalar(sdgTH[x], ps, 1.0, None,
                                                    op0=OP.mult)
                        else:
                            nc.scalar.copy(sdgTH[x], ps)
                        # staging rows for this half: s chunks + d chunks
                        nc.sync.dma_start(lhsH[x][0:1, :], sdgTH[x][0:4, :])
                        nc.sync.dma_start(rhsH[x][1:2, :], sdgTH[x][4:8, :])
                        # per-partition g columns + diag tiles for this half
                        for cc in range(NT // 2):
                            c = x * (NT // 2) + cc
                            ps12 = pps.tile([128, 12], f32, tag="sdgc")
                            nc.tensor.matmul(
                                ps12, srcTH[x][:, cc * 128:(cc + 1) * 128],
                                a_sb, start=True, stop=True)
                            if CFG["dve_copies"]:
                                nc.vector.tensor_scalar(
                                    sdg_sb[:, c * 12:(c + 1) * 12], ps12,
                                    1.0, None, op0=OP.mult)
                            else:
                                nc.scalar.copy(
                                    sdg_sb[:, c * 12:(c + 1) * 12], ps12)
                            for h in range(H):
                                g_col = sdg_sb[:, c * 12 + 8 + h:
                                               c * 12 + 9 + h]
                                eng = nc.vector if x == 0 else nc.gpsimd
                                eng.tensor_scalar(
                                    dps[c][:, h * 128:(h + 1) * 128],
                                    ident_bf, g_col, None, op0=OP.mult)
                            if CFG["pdiag_tiles"] and c < 4:
                                for h in range(H):
                                    d_col = sdg_sb[:, c * 12 + 4 + h:
                                                   c * 12 + 5 + h]
                                    nc.vector.tensor_scalar(
                                        ddp[c][:, h * 128:(h + 1) * 128],
                                        ident_bf, d_col, None, op0=OP.mult)
                                if c < 2:
                                    for h in range(H):
                                        s_col = sdg_sb[:, c * 12 + h:
                                                       c * 12 + 1 + h]
                                        nc.vector.tensor_scalar(
                                            dss[c][:, h * 128:(h + 1) * 128],
                                            ident_bf, s_col, None,
                                            op0=OP.mult)

                def lhsP(h, i0):
                    return lhsH[i0 // 512][:, h * 512 + (i0 % 512):
                                           h * 512 + (i0 % 512) + 128]

                def rhsP(h, j0):
                    return rhsH[j0 // 512][:, h * 512:(h + 1) * 512]

                psp_cm = tc.tile_pool(name="ps", bufs=2, space="PSUM")
                psp = psp_cm.__enter__()
                # j-block-major order: the first NT tiles only need rhs
                # half A and diag chunks 0..3; cnt tiles stay resident and
                # are reused by the second pass.
                tiles = [(idx, it, jb) for jb in range(N // JB)
                         for idx, it in enumerate(order)]

                # Software pipeline: stage A at tile t, B at t-1, C at t-2,
                # D at t-3.  Every instruction an engine dequeues has inputs
                # that finished >= 1 tile ago, so the in-order engine queues
                # never head-of-line block on cross-engine round trips.
                state = {}

                def stage_a(t):
                    idx, it, jb = tiles[t]
                    i0, j0 = it * 128, jb * JB
                    if jb == 0:
                        cn_t = cnt_pref.pop(idx)
                        if idx + 2 < len(order):
                            cnt_pref[idx + 2] = load_cnt(idx + 2)
                        state[("cn", idx)] = cn_t
                    cn_t = state[("cn", idx)]
                    cnt_t = cn_t[:, 0:N]
                    nct_t = cn_t[:, N:2 * N]
                    dve_route = ((t % 9) in CFG["dve_prelu"]
                                 and t < CFG["dve_prelu_until"])
                    if CFG["inplace_prelu"]:
                        # one full psa tile; prelu runs in place in PSUM and
                        # exp reads PSUM directly (no l round-trip)
                        psa = psp.tile([128, H * JB], f32, tag="psa")
                        for h in range(H):
                            nc.tensor.matmul(
                                psa[:, h * JB:(h + 1) * JB],
                                dps[it][:, h * 128:(h + 1) * 128],
                                cnt_t[:, j0:j0 + JB],
                                start=True, stop=True)
                        pv = psa.rearrange("p (h j) -> p h j", h=H)
                        for c in range(JB // 128):
                            jc = jb * (JB // 128) + c
                            nc.tensor.matmul(
                                pv[:, :, c * 128:(c + 1) * 128],
                                nct_t[:, jc * 128:(jc + 1) * 128],
                                dps[jc],
                                start=False, stop=True,
                                skip_group_check=True)
                        for h in range(H):
                            nc.tensor.matmul(
                                psa[:, h * JB:(h + 1) * JB],
                                lhsP(h, i0).bitcast(f32r),
                                rhsP(h, j0).bitcast(f32r),
                                start=False, stop=True,
                                skip_group_check=True)
                        if CFG["ablate"] >= 6:
                            return
                        if dve_route:
                            nc.vector._custom_dve(
                                prelu_op, out=psa[:, :], in0=psa,
                                imm2=NEG_SLOPE)
                        else:
                            nc.scalar.activation(psa[:, :], psa, AF.Prelu,
                                                 alpha=NEG_SLOPE)
                        state[("l", t)] = psa
                        return
                    l_t = lp.tile([128, H * JB], f32, tag="l")
                    # two half-psa tiles (heads 0-1 / 2-3, 2 banks each,
                    # 4 psum bufs): PE for tile t+2 only waits on the
                    # matching half's prelu, halving the psa-reuse stall
                    for hp in range(2):
                        psa = psp.tile([128, 2 * JB], f32, tag=f"psa{hp}")
                        for k in range(2):
                            h = hp * 2 + k
                            nc.tensor.matmul(
                                psa[:, k * JB:(k + 1) * JB],
                                dps[it][:, h * 128:(h + 1) * 128],
                                cnt_t[:, j0:j0 + JB],
                                start=True, stop=True)
                        pv = psa.rearrange("p (h j) -> p h j", h=2)
                        for c in range(JB // 128):
                            jc = jb * (JB // 128) + c
                            nc.tensor.matmul(
                                pv[:, :, c * 128:(c + 1) * 128],
                                nct_t[:, jc * 128:(jc + 1) * 128],
                                dps[jc][:, hp * 256:(hp + 1) * 256],
                                start=False, stop=True,
                                skip_group_check=True)
                        if t < CFG["pdiag_tiles"]:
                            # P via diag matmuls (bf16): s-part row-style,
                            # d-part col-style - skips the staging-DMA wait
                            for k in range(2):
                                h = hp * 2 + k
                                nc.tensor.matmul(
                                    psa[:, k * JB:(k + 1) * JB],
                                    dss[it][:, h * 128:(h + 1) * 128],
                                    ones_bf,
                                    start=False, stop=True,
                                    skip_group_check=True)
                            for c in range(JB // 128):
                                nc.tensor.matmul(
                                    pv[:, :, c * 128:(c + 1) * 128],
                                    ones_bf[:, 0:128],
                                    ddp[c][:, hp * 256:(hp + 1) * 256],
                                    start=False, stop=True,
                                    skip_group_check=True)
                        else:
                            for k in range(2):
                                h = hp * 2 + k
                                nc.tensor.matmul(
                                    psa[:, k * JB:(k + 1) * JB],
                                    lhsP(h, i0).bitcast(f32r),
                                    rhsP(h, j0).bitcast(f32r),
                                    start=False, stop=True,
                                    skip_group_check=True)
                        if CFG["ablate"] >= 6:
                            continue
                        lv = l_t[:, hp * 2 * JB:(hp + 1) * 2 * JB]
                        if CFG["half_split_prelu"]:
                            on_dve = ((hp + t) % 2 == 1 if
                                      CFG["hsp_alt"] else hp == 1)
                            if t in CFG["both_dve"]:
                                on_dve = True
                        else:
                            on_dve = dve_route
                        if on_dve:
                            nc.vector._custom_dve(
                                prelu_op, out=lv, in0=psa, imm2=NEG_SLOPE)
                        else:
                            nc.scalar.activation(lv, psa, AF.Prelu,
                                                 alpha=NEG_SLOPE)
                    if CFG["ablate"] >= 6:
                        return
                    state[("l", t)] = l_t

                def stage_b(t):
                    if CFG["ablate"] >= 5:
                        return
                    l_t = state.pop(("l", t))
                    m_t = mp.tile([128, H * JB], bf16, tag="m")
                    nc.scalar.activation(m_t, l_t, AF.Exp)
                    state[("m", t)] = m_t

                def stage_e(t):
                    if CFG["ablate"] >= 4:
                        return
                    m_t = state[("m", t)]
                    s2 = mn.tile([128, 2 * JB], bf16, tag="s2")
                    s2eng = (nc.gpsimd if ((t % 4) in CFG["pool_s2"]
                             or t >= CFG["ps2_from"]) else nc.vector)
                    s2eng.tensor_tensor(
                        s2, m_t[:, 0:2 * JB], m_t[:, 2 * JB:4 * JB],
                        op=OP.add)
                    if CFG["fuse_rsum"]:
                        state[("s2", t)] = s2
                    else:
                        s_f = mn.tile([128, JB], f32, tag="s")
                        s_eng = (nc.vector if t >= CFG["s_dve_from"]
                                 else nc.gpsimd)
                        s_eng.tensor_tensor(
                            s_f, s2[:, 0:JB], s2[:, JB:2 * JB], op=OP.add)
                        state[("s", t)] = s_f

                def stage_c(t):
                    if CFG["ablate"] >= 4 or CFG["divide"]:
                        return
                    if CFG["fuse_rsum"]:
                        s2 = state.pop(("s2", t))
                        r_b = mn.tile([128, JB], bf16, tag="rb")
                        from concourse.dve_ops import RECIP_APPROX_FAST_CONSTS
                        cc = RECIP_APPROX_FAST_CONSTS
                        nc.vector._custom_dve(
                            rsum_op, out=r_b[:, :], in0=s2[:, 0:JB],
                            in1=s2[:, JB:2 * JB], s0=cc["s0"], s1=cc["s1"])
                        state[("r", t)] = r_b
                        return
                    s_f = state.pop(("s", t))
                    if CFG["ablate"] >= 3:
                        return
                    # custom NR reciprocal computes in the f32 pipeline and
                    # converts to bf16 at the write, saving a convert pass
                    from concourse.dve_ops import (RECIP_APPROX_FAST_CONSTS,
                                                   RECIPROCAL_APPROX_FAST)
                    r_b = mn.tile([128, JB], bf16, tag="rb")
                    cc = RECIP_APPROX_FAST_CONSTS
                    nc.vector._custom_dve(
                        RECIPROCAL_APPROX_FAST, out=r_b[:, :], in0=s_f[:, :],
                        s0=cc["s0"], s1=cc["s1"], imm2=cc["imm2"])
                    state[("r", t)] = r_b

                def stage_d(t):
                    if CFG["ablate"] >= 3:
                        return
                    idx, it, jb = tiles[t]
                    i0, j0 = it * 128, jb * JB
                    m_t = state.pop(("m", t))
                    r_b = state.pop(("r", t))
                    rap = r_b[:, :]
                    r_b4 = bass.AP(tensor=rap.tensor, offset=rap.offset,
                                   ap=[rap.ap[0], [0, H], rap.ap[1]])
                    o_t = obp.tile([128, H * JB], bf16, tag="o")
                    o3 = o_t.rearrange("p (h j) -> p h j", h=H)
                    m3 = m_t.rearrange("p (h j) -> p h j", h=H)
                    if t == len(tiles) - 1 and CFG["quad_last"]:
                        # very last tile: single-plane mult+store quads for
                        # the finest overlap of the closing DMA
                        for g in range(H):
                            nc.vector.tensor_tensor(
                                o3[:, g:g + 1], m3[:, g:g + 1], r_b[:, :],
                                op=OP.mult)
                            nc.sync.dma_start(
                                out_d[g:g + 1, i0:i0 + 128, j0:j0 + JB]
                                .rearrange("h p j -> p h j"),
                                o3[:, g:g + 1])
                        return
                    if t >= len(tiles) - CFG["split_last"]:
                        # closing tiles: mult+store in plane-halves so the
                        # final DMA overlaps the second half's multiply
                        r_b2 = bass.AP(tensor=rap.tensor, offset=rap.offset,
                                       ap=[rap.ap[0], [0, 2], rap.ap[1]])
                        for g in range(2):
                            nc.vector.tensor_tensor(
                                o3[:, 2 * g:2 * g + 2],
                                m3[:, 2 * g:2 * g + 2], r_b2, op=OP.mult)
                            nc.sync.dma_start(
                                out_d[2 * g:2 * g + 2, i0:i0 + 128,
                                      j0:j0 + JB]
                                .rearrange("h p j -> p h j"),
                                o3[:, 2 * g:2 * g + 2])
                        return
                    ps = CFG["pool_mult"] if t < CFG["pm_cut"] else 0
                    if ps:
                        r_b3 = bass.AP(tensor=rap.tensor, offset=rap.offset,
                                       ap=[rap.ap[0], [0, H - ps], rap.ap[1]])
                        r_b1 = bass.AP(tensor=rap.tensor, offset=rap.offset,
                                       ap=[rap.ap[0], [0, ps], rap.ap[1]])
                        nc.vector.tensor_tensor(
                            o3[:, 0:H - ps], m3[:, 0:H - ps], r_b3, op=OP.mult)
                        nc.gpsimd.tensor_tensor(
                            o3[:, H - ps:H], m3[:, H - ps:H], r_b1, op=OP.mult)
                    else:
                        nc.vector.tensor_tensor(o3, m3, r_b4, op=OP.mult)
                    if CFG["ablate"] >= 1:
                        return
                    nc.sync.dma_start(
                        out_d[:, i0:i0 + 128, j0:j0 + JB]
                        .rearrange("h p j -> p h j"),
                        o_t.rearrange("p (h j) -> p h j", h=H))

                nt_total = len(tiles)
                stage_order = CFG.get("stage_order", "abecd")
                lags = CFG.get("lags") or {"a": 0, "b": 1, "e": 3,
                                           "c": 4, "d": 5}
                fns = {"a": stage_a, "b": stage_b, "e": stage_e,
                       "c": stage_c, "d": stage_d}
                maxlag = max(lags.values())
                for t in range(nt_total + maxlag):
                    for ch in stage_order:
                        if 0 <= t - lags[ch] < nt_total:
                            fns[ch](t - lags[ch])
                psp_cm.__exit__(None, None, None)
    nc.finalize()
    return nc


def _prepare_in_maps(src, edge_index, W_lin, a_src, a_dst, W_edge, a_edge):
    import ml_dtypes

    src = np.ascontiguousarray(np.asarray(src, dtype=np.float32))
    ei = np.asarray(edge_index).astype(np.int64)
    W_lin = np.asarray(W_lin, dtype=np.float32)
    a_src = np.asarray(a_src, dtype=np.float32)
    a_dst = np.asarray(a_dst, dtype=np.float32)
    W_edge = np.asarray(W_edge, dtype=np.float32)
    a_edge = np.asarray(a_edge, dtype=np.float32)

    # fold weights: A = [W_lin@a_src | W_lin@a_dst | W_edge@a_edge]  [128,12]
    A = np.concatenate(
        [W_lin @ a_src, W_lin @ a_dst, W_edge @ a_edge], axis=1
    ).astype(np.float32)
    # edge multiplicity matrix (shared across batches)
    cnt = np.zeros((N, N), np.float32)
    np.add.at(cnt, (ei[0], ei[1]), 1.0)
    # -cnt^T packed per-i-tile: ncntT[it*128+p, q*128+f] = -cnt[it*128+f, q*128+p]
    T = np.ascontiguousarray((-cnt).T)          # T[j, i] = -cnt[i, j]
    ncntT = T.reshape(NT, 128, NT, 128).transpose(2, 1, 0, 3).reshape(N, N)
    cn = np.ascontiguousarray(
        np.concatenate([cnt, ncntT], axis=0)).astype(ml_dtypes.bfloat16)
    return [
        {"src": np.ascontiguousarray(src[b]), "cn": cn, "A": A}
        for b in range(B)
    ]


def kernel(src, edge_index, W_lin, a_src, a_dst, W_edge, a_edge):
    from concourse.bass_utils import run_bass_kernel_spmd

    in_maps = _prepare_in_maps(src, edge_index, W_lin, a_src, a_dst,
                               W_edge, a_edge)
    nc = _build_nc()
    res = run_bass_kernel_spmd(nc, in_maps, core_ids=list(range(B)))
    out = np.stack(
        [np.asarray(res.results[b]["out"]).astype(np.float32)
         .transpose(1, 2, 0) for b in range(B)], axis=0)
    return np.ascontiguousarray(out)


if __name__ == "__main__":
    rng = np.random.default_rng(0)
    inputs = {
        "src": rng.standard_normal((B, N, F_IN), dtype=np.float32),
        "edge_index": rng.integers(0, N, (2, 32768)).astype(np.int32),
        "W_lin": rng.standard_normal((F_IN, 128), dtype=np.float32) / np.sqrt(F_IN),
        "a_src": rng.standard_normal((128, H), dtype=np.float32) / np.sqrt(128),
        "a_dst": rng.standard_normal((128, H), dtype=np.float32) / np.sqrt(128),
        "W_edge": rng.standard_normal((F_IN, 64), dtype=np.float32) / np.sqrt(F_IN),
        "a_edge": rng.standard_normal((64, H), dtype=np.float32) / np.sqrt(64),
    }
    out = kernel(**inputs)
    print("out", out.shape, out.dtype, out.sum())
